# revision 58
# baseline (speedup 1.0000x reference)
"""Trainium2 Bass kernel for per-token outer-product softmax attention.

Reference computation (per token t of 1600, H=256):
    k = tanh(x W0 + b0);  q = tanh(x W1 + b1)
    scores[i,j] = k[i]*q[j];  attn = softmax_j(scores);  out = attn @ x

Key algebra: k,q are tanh outputs so k[i]*q[j] in (-1,1). On [-1,1],
exp(s) is approximated to fp32-noise level by a low-degree polynomial
P(s) = sum_d c_d s^d, and P(k_i q_j) = sum_d c_d k_i^d q_j^d is
SEPARABLE. Softmax numerator/denominator become per-token moments:
    num_i = sum_d (c_d sum_j q_j^d x_j) k_i^d
    den_i = sum_d (c_d sum_j q_j^d)     k_i^d
so the 256x256 scores tensor is never materialized. Per 128-token tile
this is ~2D fused multiply+reduce passes (moments) plus two Estrin
polynomial evaluations, all [128,256] vector instructions, spread
across DVE / GpSimd(Pool) / ACT engines. The queries matmul+tanh is
scheduled before the keys one so the moment pipeline starts ASAP.

Sharding: pure data parallel over tokens, 200 tokens/core x 8 cores;
weights replicated.
"""

import numpy as np
from contextlib import ExitStack

import concourse.bass as bass
import concourse.bacc as bacc
import concourse.tile as tile
from concourse import mybir
from concourse.bass_utils import run_bass_kernel_spmd
from concourse.masks import make_identity

F32 = mybir.dt.float32
AF = mybir.ActivationFunctionType
OP = mybir.AluOpType

B, S, M, H = 4, 10, 40, 256
T = B * S * M            # 1600 tokens
NCORES = 8
TC = T // NCORES         # 200 tokens per core
BLOCKS = [(0, 128), (128, TC - 128)]

# Chebyshev-interpolation coefficients (monomial basis) of exp on [-1,1].
# Max rel err: D=6 -> 7.7e-6, D=8 -> 2.7e-8.
COEFS = {
    6: [1.0, 1.000022235, 0.5000027659, 0.1664890938, 0.04164456983,
        0.008686644402, 0.001432899535],
    8: [1.0, 0.9999999011, 0.4999999901, 0.1666679842, 0.04166679799,
        0.008328598904, 0.001388416857, 0.0002046983349, 2.542872193e-05],
}

D = 6

# Engine assignment knobs (tuned against real-HW loop benchmarks):
CFG = {
    "n_den_act": 6,     # denominator accums d=2..D: first n on ACT, rest DVE TS+accum
    "n_num_pool": 0,    # numerator moments d=2..D: first n via Pool TT + ACT accum
    "chain_tt_pool": 3,  # estrin only: of the 12 combine-TTs, how many on Pool
    "pairs_act": 8,     # estrin only: of the 8 pairs per block, how many on ACT
    "j0_act": True,     # d=0 numerator moment on ACT instead of DVE
    "tree_dve": 0,      # of the QP-tree TTs, how many on DVE instead of Pool
    "kpow_dve": 0,      # estrin only: of the 3 K-power TTs, how many on DVE
    "x_dma": "sync",    # engine for X loads: sync | scalar | gpsimd
    "out_dma": "sync",  # engine for output stores
    "recip": "approx",  # approx (~2 ULP custom DVE) | exact
    "scrp_bufs": 8,
    "phase_limit": 4,   # 0=min body, 1=KQ only, 2=+moments, 3=+chains, 4=full
    "chain_mode": "horner_dve",  # estrin | horner_dve | horner_mix
}


def _pow_tree(dmax):
    """Return list of (d, a, b) meaning QP_d = QP_a * QP_b, log-depth order."""
    steps = []
    have = {1}
    for d in range(2, dmax + 1):
        a = d // 2
        b = d - a
        steps.append((d, a, b))
        have.add(d)
    return steps


def build_kernel(reps: int = 1) -> bass.Bass:
    coef = COEFS[D]
    # wcat columns: [W1lo|W1hi|biasQ|coef || W0lo|W0hi|biasK]
    WQ = 2 * H + H + 2 * (D + 1)   # 786
    WK = 2 * H + H                 # 768
    WEXT = WQ + WK
    nc = bacc.Bacc("TRN2", target_bir_lowering=False, debug=False)
    xs = nc.declare_dram_parameter("xs", [TC, H], F32, isOutput=False)
    wcat = nc.declare_dram_parameter("wcat", [128, WEXT], F32, isOutput=False)
    out = nc.declare_dram_parameter("out", [TC, H], F32, isOutput=True)

    with tile.TileContext(nc) as tc, ExitStack() as ctx:
        consts = ctx.enter_context(tc.tile_pool(name="consts", bufs=1))
        io = ctx.enter_context(tc.tile_pool(name="io", bufs=2))
        work = ctx.enter_context(tc.tile_pool(name="work", bufs=2))
        pows = ctx.enter_context(tc.tile_pool(name="pows", bufs=2))
        scrp = ctx.enter_context(tc.tile_pool(name="scrp", bufs=CFG.get("scrp_bufs", 3)))
        mom = ctx.enter_context(tc.tile_pool(name="mom", bufs=2))
        psT = ctx.enter_context(tc.tile_pool(name="psT", bufs=2, space="PSUM"))
        psKQ = ctx.enter_context(tc.tile_pool(name="psKQ", bufs=2, space="PSUM"))

        x_eng = getattr(nc, CFG["x_dma"])
        out_eng = getattr(nc, CFG["out_dma"])
        # Small constants first on the Pool queue, then X (gates the whole
        # pipeline), then the Q-side weights (gate MM-Q), then K-side.
        ident = consts.tile([128, 128], F32)
        make_identity(nc, ident)
        ones1 = consts.tile([1, 128], F32)
        nc.gpsimd.memset(ones1, 1.0)
        Xs = []
        for t0, tl in BLOCKS:
            X = io.tile([128, H], F32, tag=f"X{t0}")
            x_eng.dma_start(out=X[:tl, :], in_=xs[t0 : t0 + tl, :])
            Xs.append(X)
        wallQ = consts.tile([128, WQ], F32)
        nc.gpsimd.dma_start(out=wallQ, in_=wcat[:, 0:WQ])
        wallK = consts.tile([128, WK], F32)
        nc.gpsimd.dma_start(out=wallK, in_=wcat[:, WQ:WEXT])
        bsbQ = wallQ[0:1, 2 * H : 3 * H]
        bsbK = wallK[0:1, 2 * H : 3 * H]
        ctile = wallQ[:, 3 * H : 3 * H + 2 * (D + 1)].rearrange(
            "p (two d) -> p two d", two=2
        )

        def body():
            if CFG["phase_limit"] == 0:
                for t0, tl in BLOCKS:
                    O = io.tile([128, H], F32, tag="O")
                    nc.vector.tensor_copy(O[:tl, :], Xs[0][:tl, :])
                    out_eng.dma_start(out=out[t0 : t0 + tl, :], in_=O[:tl, :])
                return
            for bi, (t0, tl) in enumerate(BLOCKS):
                X = Xs[bi]

                # ---- x^T via PE transpose (matmul contracts over partitions)
                pT0 = psT.tile([128, 128], F32, tag="pT0")
                pT1 = psT.tile([128, 128], F32, tag="pT1")
                nc.tensor.transpose(pT0[:, :tl], X[:tl, 0:128], ident[:tl, :tl])
                nc.tensor.transpose(pT1[:, :tl], X[:tl, 128:256], ident[:tl, :tl])
                xT = work.tile([128, 256], F32, tag="xT")
                nc.vector.tensor_copy(xT[:, 0:tl], pT0[:, :tl])
                nc.scalar.copy(xT[:, 128 : 128 + tl], pT1[:, :tl])

                # ---- queries first: moments only need Q and X.
                # Bias matmul leads: it only needs constants, so it runs
                # during the xT dependency chain.
                psQ = psKQ.tile([128, H], F32, tag="psQ")
                nc.tensor.matmul(
                    psQ[:tl, :], ones1[:, :tl], bsbQ,
                    start=True, stop=False,
                )
                nc.tensor.matmul(
                    psQ[:tl, :], xT[:, 0:tl], wallQ[:, 0:256],
                    start=False, stop=False,
                )
                nc.tensor.matmul(
                    psQ[:tl, :], xT[:, 128 : 128 + tl], wallQ[:, 256:512],
                    start=False, stop=True,
                )
                # Smom[:, 0, :] = raw numerator moments, [:, 1, :] = denominator
                Smom = mom.tile([128, 2, D + 1], F32, tag="Smom")
                nc.gpsimd.memset(Smom[:tl, 1, 0:1], float(H))
                Qt = work.tile([128, H], F32, tag="Qt")
                nc.scalar.activation(
                    Qt[:tl, :], psQ[:tl, :], AF.Tanh,
                    accum_out=Smom[:tl, 1, 1:2],
                )
                Q = Qt[:tl, :]

                # ---- keys (overlaps with the moment pipeline below)
                psK = psKQ.tile([128, H], F32, tag="psK")
                nc.tensor.matmul(
                    psK[:tl, :], ones1[:, :tl], bsbK,
                    start=True, stop=False,
                )
                nc.tensor.matmul(
                    psK[:tl, :], xT[:, 0:tl], wallK[:, 0:256],
                    start=False, stop=False,
                )
                nc.tensor.matmul(
                    psK[:tl, :], xT[:, 128 : 128 + tl], wallK[:, 256:512],
                    start=False, stop=True,
                )
                Kt = work.tile([128, H], F32, tag="Kt")
                nc.scalar.activation(Kt[:tl, :], psK[:tl, :], AF.Tanh)
                K = Kt[:tl, :]

                if CFG["phase_limit"] == 1:
                    O = io.tile([128, H], F32, tag="O")
                    nc.vector.tensor_add(O[:tl, :], Qt[:tl, :], Kt[:tl, :])
                    out_eng.dma_start(out=out[t0 : t0 + tl, :], in_=O[:tl, :])
                    continue

                # ---- raw moments (unscaled powers QP_d = q^d)
                j0 = scrp.tile([128, H], F32, tag="scr")
                if CFG["j0_act"]:
                    nc.scalar.activation(
                        j0[:tl, :], X[:tl, :], AF.Identity,
                        accum_out=Smom[:tl, 0, 0:1],
                    )
                else:
                    nc.vector.tensor_scalar(
                        out=j0[:tl, :], in0=X[:tl, :], scalar1=1.0, scalar2=0.0,
                        op0=OP.mult, op1=OP.add, accum_out=Smom[:tl, 0, 0:1],
                    )
                s1 = scrp.tile([128, H], F32, tag="scr")
                nc.vector.scalar_tensor_tensor(
                    out=s1[:tl, :], in0=Q, scalar=1.0, in1=X[:tl, :],
                    op0=OP.mult, op1=OP.mult, accum_out=Smom[:tl, 0, 1:2],
                )
                QP = {1: Q}
                n_act = 0
                n_pool = 0
                n_tree_dve = 0
                for d, a, b in _pow_tree(D):
                    QPn = pows.tile([128, H], F32, tag=f"qp{d}")
                    if n_tree_dve < CFG["tree_dve"]:
                        n_tree_dve += 1
                        nc.vector.tensor_mul(QPn[:tl, :], QP[a], QP[b])
                    else:
                        nc.gpsimd.tensor_mul(QPn[:tl, :], QP[a], QP[b])
                    QP[d] = QPn[:tl, :]
                    # denominator accum
                    if n_act < CFG["n_den_act"]:
                        n_act += 1
                        ja = scrp.tile([128, H], F32, tag="scr")
                        nc.scalar.activation(
                            ja[:tl, :], QPn[:tl, :], AF.Identity,
                            accum_out=Smom[:tl, 1, d : d + 1],
                        )
                    elif CFG.get("den_dve_op", "ts") == "ts":
                        jr = scrp.tile([128, H], F32, tag="scr")
                        nc.vector.tensor_scalar(
                            out=jr[:tl, :], in0=QPn[:tl, :], scalar1=1.0,
                            scalar2=0.0, op0=OP.mult, op1=OP.add,
                            accum_out=Smom[:tl, 1, d : d + 1],
                        )
                    else:
                        nc.vector.tensor_reduce(
                            out=Smom[:tl, 1, d : d + 1], in_=QPn[:tl, :],
                            axis=mybir.AxisListType.X, op=OP.add,
                        )
                    # numerator moment: sum (q^d * x)
                    if n_pool < CFG["n_num_pool"]:
                        n_pool += 1
                        sd = scrp.tile([128, H], F32, tag="scr")
                        nc.gpsimd.tensor_mul(sd[:tl, :], QPn[:tl, :], X[:tl, :])
                        jb = scrp.tile([128, H], F32, tag="scr")
                        nc.scalar.activation(
                            jb[:tl, :], sd[:tl, :], AF.Identity,
                            accum_out=Smom[:tl, 0, d : d + 1],
                        )
                    else:
                        sd = scrp.tile([128, H], F32, tag="scr")
                        nc.vector.scalar_tensor_tensor(
                            out=sd[:tl, :], in0=QPn[:tl, :], scalar=1.0,
                            in1=X[:tl, :], op0=OP.mult, op1=OP.mult,
                            accum_out=Smom[:tl, 0, d : d + 1],
                        )

                # ---- scale moments by polynomial coefficients (one tiny TT)
                A2 = mom.tile([128, 2, D + 1], F32, tag="A2")
                nc.vector.tensor_mul(A2[:tl, :, :], Smom[:tl, :, :], ctile[:tl, :, :])

                if CFG["phase_limit"] == 2:
                    O = io.tile([128, H], F32, tag="O")
                    nc.vector.tensor_copy(O[:tl, :], K)
                    nc.vector.tensor_scalar(
                        out=O[:tl, 0 : 2 * (D + 1)],
                        in0=A2[:tl, :, :].rearrange("p a b -> p (a b)"),
                        scalar1=1.0, scalar2=None, op0=OP.mult,
                    )
                    out_eng.dma_start(out=out[t0 : t0 + tl, :], in_=O[:tl, :])
                    continue

                # ---- K powers for Estrin: k^2, k^4, k^8
                if CFG["chain_mode"] == "estrin":
                    kp_engs = [nc.vector] * CFG["kpow_dve"] + [nc.gpsimd] * 3
                    K2 = pows.tile([128, H], F32, tag="K2")
                    kp_engs[0].tensor_mul(K2[:tl, :], K, K)
                    K4 = pows.tile([128, H], F32, tag="K4")
                    kp_engs[1].tensor_mul(K4[:tl, :], K2[:tl, :], K2[:tl, :])
                    K8 = pows.tile([128, H], F32, tag="K8")
                    kp_engs[2].tensor_mul(K8[:tl, :], K4[:tl, :], K4[:tl, :])

                # ---- Estrin evaluation of both polynomials over K
                # P(k) = (a0 + a1 k) + k^2 (a2 + a3 k)
                #      + k^4 [(a4 + a5 k) + k^2 (a6 + a7 k)] + a8 k^8
                cnt = {"pair": 0, "tt": 0}

                def estrin(which, tag):
                    a = lambda d: A2[:tl, which, d : d + 1]
                    ps = []
                    for i in range(4):
                        p = scrp.tile([128, H], F32, tag=f"p{tag}{i}")
                        if cnt["pair"] < CFG["pairs_act"]:
                            cnt["pair"] += 1
                            nc.scalar.activation(
                                p[:tl, :], K, AF.Identity,
                                scale=a(2 * i + 1), bias=a(2 * i),
                            )
                        else:
                            nc.vector.tensor_scalar(
                                out=p[:tl, :], in0=K, scalar1=a(2 * i + 1),
                                scalar2=a(2 * i), op0=OP.mult, op1=OP.add,
                            )
                        ps.append(p)
                    n_pool_tt = CFG["chain_tt_pool"]
                    engs = []
                    for _ in range(6):
                        engs.append(
                            nc.gpsimd if cnt["tt"] < n_pool_tt else nc.vector
                        )
                        cnt["tt"] += 1
                    t1 = scrp.tile([128, H], F32, tag=f"t1{tag}")
                    engs[0].tensor_mul(t1[:tl, :], ps[1][:tl, :], K2[:tl, :])
                    e01 = scrp.tile([128, H], F32, tag=f"e01{tag}")
                    engs[1].tensor_add(e01[:tl, :], t1[:tl, :], ps[0][:tl, :])
                    t2 = scrp.tile([128, H], F32, tag=f"t2{tag}")
                    engs[2].tensor_mul(t2[:tl, :], ps[3][:tl, :], K2[:tl, :])
                    e23 = scrp.tile([128, H], F32, tag=f"e23{tag}")
                    engs[3].tensor_add(e23[:tl, :], t2[:tl, :], ps[2][:tl, :])
                    t3 = scrp.tile([128, H], F32, tag=f"t3{tag}")
                    engs[4].tensor_mul(t3[:tl, :], e23[:tl, :], K4[:tl, :])
                    f = scrp.tile([128, H], F32, tag=f"f{tag}")
                    engs[5].tensor_add(f[:tl, :], t3[:tl, :], e01[:tl, :])
                    res = work.tile([128, H], F32, tag=f"res{tag}")
                    nc.vector.scalar_tensor_tensor(
                        out=res[:tl, :], in0=K8[:tl, :], scalar=a(8),
                        in1=f[:tl, :], op0=OP.mult, op1=OP.add,
                    )
                    return res

                def horner_chain(which, tag, add_eng, mul_eng):
                    # u = a_D k; repeat: u = (u + a_d) * k; final +a_0
                    a = lambda d: A2[:tl, which, d : d + 1]
                    u = work.tile([128, H], F32, tag=f"res{tag}")
                    nc.vector.tensor_scalar(
                        out=u[:tl, :], in0=K, scalar1=a(D), scalar2=None,
                        op0=OP.mult,
                    )
                    for d in range(D - 1, 0, -1):
                        if add_eng is None:
                            nc.vector.scalar_tensor_tensor(
                                out=u[:tl, :], in0=u[:tl, :], scalar=a(d),
                                in1=K, op0=OP.add, op1=OP.mult,
                            )
                        else:
                            add_eng(u, a(d))
                            mul_eng.tensor_mul(u[:tl, :], u[:tl, :], K)
                    nc.vector.tensor_scalar(
                        out=u[:tl, :], in0=u[:tl, :], scalar1=a(0),
                        scalar2=None, op0=OP.add,
                    )
                    return u

                mode = CFG["chain_mode"]
                if mode == "estrin":
                    uN = estrin(0, "n")
                    uD = estrin(1, "d")
                elif mode == "horner_dve":
                    uN = horner_chain(0, "n", None, None)
                    uD = horner_chain(1, "d", None, None)
                else:  # horner_mix: numerator on DVE, denominator ACT/Pool
                    uN = horner_chain(0, "n", None, None)

                    def act_add(u, aap):
                        nc.scalar.activation(
                            out=u[:tl, :], in_=u[:tl, :], func=AF.Identity,
                            bias=aap,
                        )

                    uD = horner_chain(1, "d", act_add, nc.gpsimd)

                if CFG["phase_limit"] == 3:
                    O = io.tile([128, H], F32, tag="O")
                    nc.vector.tensor_add(O[:tl, :], uN[:tl, :], uD[:tl, :])
                    out_eng.dma_start(out=out[t0 : t0 + tl, :], in_=O[:tl, :])
                    continue

                # ---- out = num / den
                rD = work.tile([128, H], F32, tag="rD")
                if CFG["recip"] == "approx":
                    rs = scrp.tile([128, H], F32, tag="scr")
                    nc.vector.reciprocal_approx_accurate(
                        rD[:tl, :], uD[:tl, :], rs[:tl, :]
                    )
                else:
                    nc.vector.reciprocal(rD[:tl, :], uD[:tl, :])
                O = io.tile([128, H], F32, tag="O")
                nc.gpsimd.tensor_mul(O[:tl, :], uN[:tl, :], rD[:tl, :])
                out_eng.dma_start(out=out[t0 : t0 + tl, :], in_=O[:tl, :])

        if reps == 1:
            body()
        else:
            with tc.For_i(0, reps, 1):
                body()

    nc.compile()
    return nc


_NC = None


def _get_nc():
    global _NC
    if _NC is None:
        _NC = build_kernel()
    return _NC


def _make_in_maps(x, W0, b0, W1, b1):
    coef = COEFS[D]
    xf = np.ascontiguousarray(np.asarray(x, np.float32).reshape(T, H))
    W0 = np.asarray(W0, np.float32)
    W1 = np.asarray(W1, np.float32)
    biasQ = np.zeros((128, H), np.float32)
    biasQ[0, :] = np.asarray(b1, np.float32)
    biasK = np.zeros((128, H), np.float32)
    biasK[0, :] = np.asarray(b0, np.float32)
    c2 = np.tile(
        np.array(coef + coef, np.float32).reshape(1, 2 * (D + 1)), (128, 1)
    )
    wcat = np.ascontiguousarray(
        np.concatenate(
            [W1[:128, :], W1[128:, :], biasQ, c2,
             W0[:128, :], W0[128:, :], biasK],
            axis=1,
        )
    )  # [128, WQ+WK]
    return [
        {
            "xs": np.ascontiguousarray(xf[c * TC : (c + 1) * TC]),
            "wcat": wcat,
        }
        for c in range(NCORES)
    ]


def _run(x, W0, b0, W1, b1, trace=False, **kw):
    res = run_bass_kernel_spmd(
        _get_nc(), _make_in_maps(x, W0, b0, W1, b1), list(range(NCORES)),
        trace=trace, **kw,
    )
    outs = [res.results[c]["out"] for c in range(NCORES)]
    full = np.concatenate(outs, axis=0).reshape(B, S, M, H).astype(np.float32)
    return full, res


def kernel(x, W0, b0, W1, b1):
    full, _ = _run(x, W0, b0, W1, b1, trace=False)
    return full


# revision 64
# speedup vs baseline: 1.0925x; 1.0925x over previous
"""Trainium2 Bass kernel for per-token outer-product softmax attention.

Reference computation (per token t of 1600, H=256):
    k = tanh(x W0 + b0);  q = tanh(x W1 + b1)
    scores[i,j] = k[i]*q[j];  attn = softmax_j(scores);  out = attn @ x

Key algebra: k,q are tanh outputs so k[i]*q[j] in (-1,1). On [-1,1],
exp(s) is approximated to fp32-noise level by a low-degree polynomial
P(s) = sum_d c_d s^d, and P(k_i q_j) = sum_d c_d k_i^d q_j^d is
SEPARABLE. Softmax numerator/denominator become per-token moments:
    num_i = sum_d (c_d sum_j q_j^d x_j) k_i^d
    den_i = sum_d (c_d sum_j q_j^d)     k_i^d
so the 256x256 scores tensor is never materialized. Per 128-token tile
this is ~2D fused multiply+reduce passes (moments, via
scalar_tensor_tensor accum_out) plus two fused Horner chains over k,
all [128,256] vector instructions spread across DVE / GpSimd(Pool) /
ACT engines. The queries matmul+tanh is scheduled before the keys one
so the moment pipeline starts ASAP; the final +a0 of the numerator
chain is fused with the divide.

Sharding: pure data parallel over tokens, 200 tokens/core x 8 cores;
weights replicated.
"""

import numpy as np
from contextlib import ExitStack

import concourse.bass as bass
import concourse.bacc as bacc
import concourse.tile as tile
from concourse import mybir
from concourse.bass_utils import run_bass_kernel_spmd
from concourse.masks import make_identity

F32 = mybir.dt.float32
AF = mybir.ActivationFunctionType
OP = mybir.AluOpType

B, S, M, H = 4, 10, 40, 256
T = B * S * M            # 1600 tokens
NCORES = 8
TC = T // NCORES         # 200 tokens per core
BLOCKS = [(0, 128), (128, TC - 128)]

# Chebyshev-interpolation coefficients (monomial basis) of exp on [-1,1].
# Max rel err: D=6 -> 7.7e-6, D=8 -> 2.7e-8.
COEFS = {
    6: [1.0, 1.000022235, 0.5000027659, 0.1664890938, 0.04164456983,
        0.008686644402, 0.001432899535],
    8: [1.0, 0.9999999011, 0.4999999901, 0.1666679842, 0.04166679799,
        0.008328598904, 0.001388416857, 0.0002046983349, 2.542872193e-05],
}

D = 6

# Engine assignment knobs (tuned against real-HW loop benchmarks):
CFG = {
    "n_den_act": 6,     # denominator accums d=2..D: first n on ACT, rest DVE TS+accum
    "n_num_pool": 0,    # numerator moments d=2..D: first n via Pool TT + ACT accum
    "chain_tt_pool": 3,  # estrin only: of the 12 combine-TTs, how many on Pool
    "pairs_act": 8,     # estrin only: of the 8 pairs per block, how many on ACT
    "j0_act": True,     # d=0 numerator moment on ACT instead of DVE
    "tree_dve": 0,      # of the QP-tree TTs, how many on DVE instead of Pool
    "kpow_dve": 0,      # estrin only: of the 3 K-power TTs, how many on DVE
    "x_dma": "sync",    # engine for X loads: sync | scalar | gpsimd
    "out_dma": "sync",  # engine for output stores
    "recip": "approx",  # approx (~2 ULP custom DVE) | exact
    "scrp_bufs": 8,
    "phase_limit": 4,   # 0=min body, 1=KQ only, 2=+moments, 3=+chains, 4=full
    "chain_mode": "horner_dve",  # estrin | horner_dve | horner_mix
}


def _pow_tree(dmax):
    """Return list of (d, a, b) meaning QP_d = QP_a * QP_b, log-depth order."""
    steps = []
    have = {1}
    for d in range(2, dmax + 1):
        a = d // 2
        b = d - a
        steps.append((d, a, b))
        have.add(d)
    return steps


def build_kernel(reps: int = 1, with_bias: bool = True) -> bass.Bass:
    coef = COEFS[D]
    # wcat columns: [W1lo|W1hi|biasQ|coef || W0lo|W0hi|biasK]
    WQ = 2 * H + H + 2 * (D + 1)   # 786
    WK = 2 * H + H                 # 768
    WEXT = WQ + WK
    nc = bacc.Bacc("TRN2", target_bir_lowering=False, debug=False)
    xs = nc.declare_dram_parameter("xs", [TC, H], F32, isOutput=False)
    wcat = nc.declare_dram_parameter("wcat", [128, WEXT], F32, isOutput=False)
    out = nc.declare_dram_parameter("out", [TC, H], F32, isOutput=True)

    with tile.TileContext(nc) as tc, ExitStack() as ctx:
        consts = ctx.enter_context(tc.tile_pool(name="consts", bufs=1))
        io = ctx.enter_context(tc.tile_pool(name="io", bufs=2))
        work = ctx.enter_context(tc.tile_pool(name="work", bufs=2))
        pows = ctx.enter_context(tc.tile_pool(name="pows", bufs=2))
        scrp = ctx.enter_context(tc.tile_pool(name="scrp", bufs=CFG.get("scrp_bufs", 3)))
        mom = ctx.enter_context(tc.tile_pool(name="mom", bufs=2))
        psT = ctx.enter_context(tc.tile_pool(name="psT", bufs=2, space="PSUM"))
        psKQ = ctx.enter_context(tc.tile_pool(name="psKQ", bufs=2, space="PSUM"))

        x_eng = getattr(nc, CFG["x_dma"])
        out_eng = getattr(nc, CFG["out_dma"])
        # Small constants first on the Pool queue, then X (gates the whole
        # pipeline), then the Q-side weights (gate MM-Q), then K-side.
        ident = consts.tile([128, 128], F32)
        make_identity(nc, ident)
        ones1 = consts.tile([1, 128], F32)
        nc.gpsimd.memset(ones1, 1.0)
        Xs = []
        for t0, tl in BLOCKS:
            X = io.tile([128, H], F32, tag=f"X{t0}")
            x_eng.dma_start(out=X[:tl, :], in_=xs[t0 : t0 + tl, :])
            Xs.append(X)
        wallQ = consts.tile([128, WQ], F32)
        nc.gpsimd.dma_start(out=wallQ, in_=wcat[:, 0:WQ])
        wallK = consts.tile([128, WK], F32)
        nc.gpsimd.dma_start(out=wallK, in_=wcat[:, WQ:WEXT])
        bsbQ = wallQ[0:1, 2 * H : 3 * H]
        bsbK = wallK[0:1, 2 * H : 3 * H]
        ctile = wallQ[:, 3 * H : 3 * H + 2 * (D + 1)].rearrange(
            "p (two d) -> p two d", two=2
        )

        def body():
            if CFG["phase_limit"] == 0:
                for t0, tl in BLOCKS:
                    O = io.tile([128, H], F32, tag="O")
                    nc.vector.tensor_copy(O[:tl, :], Xs[0][:tl, :])
                    out_eng.dma_start(out=out[t0 : t0 + tl, :], in_=O[:tl, :])
                return
            for bi, (t0, tl) in enumerate(BLOCKS):
                X = Xs[bi]

                # ---- x^T via PE transpose (matmul contracts over partitions)
                pT = psT.tile([128, 2, 128], F32, tag="pT")
                nc.tensor.transpose(pT[:, 0, :tl], X[:tl, 0:128], ident[:tl, :tl])
                nc.tensor.transpose(pT[:, 1, :tl], X[:tl, 128:256], ident[:tl, :tl])
                xT = work.tile([128, 2, 128], F32, tag="xT")
                nc.vector.tensor_copy(xT[:, :, :tl], pT[:, :, :tl])

                # ---- queries first: moments only need Q and X.
                # Bias matmul leads: it only needs constants, so it runs
                # during the xT dependency chain.
                psQ = psKQ.tile([128, H], F32, tag="psQ")
                if with_bias:
                    nc.tensor.matmul(
                        psQ[:tl, :], ones1[:, :tl], bsbQ,
                        start=True, stop=False,
                    )
                nc.tensor.matmul(
                    psQ[:tl, :], xT[:, 0, :tl], wallQ[:, 0:256],
                    start=not with_bias, stop=False,
                )
                nc.tensor.matmul(
                    psQ[:tl, :], xT[:, 1, :tl], wallQ[:, 256:512],
                    start=False, stop=True,
                )
                # Smom[:, 0, :] = raw numerator moments, [:, 1, :] = denominator
                Smom = mom.tile([128, 2, D + 1], F32, tag="Smom")
                nc.gpsimd.memset(Smom[:tl, 1, 0:1], float(H))
                Qt = work.tile([128, H], F32, tag="Qt")
                nc.scalar.activation(
                    Qt[:tl, :], psQ[:tl, :], AF.Tanh,
                    accum_out=Smom[:tl, 1, 1:2],
                )
                Q = Qt[:tl, :]

                # ---- keys (overlaps with the moment pipeline below)
                psK = psKQ.tile([128, H], F32, tag="psK")
                if with_bias:
                    nc.tensor.matmul(
                        psK[:tl, :], ones1[:, :tl], bsbK,
                        start=True, stop=False,
                    )
                nc.tensor.matmul(
                    psK[:tl, :], xT[:, 0, :tl], wallK[:, 0:256],
                    start=not with_bias, stop=False,
                )
                nc.tensor.matmul(
                    psK[:tl, :], xT[:, 1, :tl], wallK[:, 256:512],
                    start=False, stop=True,
                )
                Kt = work.tile([128, H], F32, tag="Kt")
                nc.scalar.activation(Kt[:tl, :], psK[:tl, :], AF.Tanh)
                K = Kt[:tl, :]

                if CFG["phase_limit"] == 1:
                    O = io.tile([128, H], F32, tag="O")
                    nc.vector.tensor_add(O[:tl, :], Qt[:tl, :], Kt[:tl, :])
                    out_eng.dma_start(out=out[t0 : t0 + tl, :], in_=O[:tl, :])
                    continue

                # ---- raw moments (unscaled powers QP_d = q^d)
                j0 = scrp.tile([128, H], F32, tag="scr")
                if CFG["j0_act"]:
                    nc.scalar.activation(
                        j0[:tl, :], X[:tl, :], AF.Identity,
                        accum_out=Smom[:tl, 0, 0:1],
                    )
                else:
                    nc.vector.tensor_scalar(
                        out=j0[:tl, :], in0=X[:tl, :], scalar1=1.0, scalar2=0.0,
                        op0=OP.mult, op1=OP.add, accum_out=Smom[:tl, 0, 0:1],
                    )
                s1 = scrp.tile([128, H], F32, tag="scr")
                nc.vector.scalar_tensor_tensor(
                    out=s1[:tl, :], in0=Q, scalar=1.0, in1=X[:tl, :],
                    op0=OP.mult, op1=OP.mult, accum_out=Smom[:tl, 0, 1:2],
                )
                QP = {1: Q}
                n_act = 0
                n_pool = 0
                n_tree_dve = 0
                for d, a, b in _pow_tree(D):
                    QPn = pows.tile([128, H], F32, tag=f"qp{d}")
                    if n_tree_dve < CFG["tree_dve"]:
                        n_tree_dve += 1
                        nc.vector.tensor_mul(QPn[:tl, :], QP[a], QP[b])
                    else:
                        nc.gpsimd.tensor_mul(QPn[:tl, :], QP[a], QP[b])
                    QP[d] = QPn[:tl, :]
                    # denominator accum
                    if n_act < CFG["n_den_act"]:
                        n_act += 1
                        ja = scrp.tile([128, H], F32, tag="scr")
                        nc.scalar.activation(
                            ja[:tl, :], QPn[:tl, :], AF.Identity,
                            accum_out=Smom[:tl, 1, d : d + 1],
                        )
                    elif CFG.get("den_dve_op", "ts") == "ts":
                        jr = scrp.tile([128, H], F32, tag="scr")
                        nc.vector.tensor_scalar(
                            out=jr[:tl, :], in0=QPn[:tl, :], scalar1=1.0,
                            scalar2=0.0, op0=OP.mult, op1=OP.add,
                            accum_out=Smom[:tl, 1, d : d + 1],
                        )
                    else:
                        nc.vector.tensor_reduce(
                            out=Smom[:tl, 1, d : d + 1], in_=QPn[:tl, :],
                            axis=mybir.AxisListType.X, op=OP.add,
                        )
                    # numerator moment: sum (q^d * x)
                    if n_pool < CFG["n_num_pool"]:
                        n_pool += 1
                        sd = scrp.tile([128, H], F32, tag="scr")
                        nc.gpsimd.tensor_mul(sd[:tl, :], QPn[:tl, :], X[:tl, :])
                        jb = scrp.tile([128, H], F32, tag="scr")
                        nc.scalar.activation(
                            jb[:tl, :], sd[:tl, :], AF.Identity,
                            accum_out=Smom[:tl, 0, d : d + 1],
                        )
                    else:
                        sd = scrp.tile([128, H], F32, tag="scr")
                        nc.vector.scalar_tensor_tensor(
                            out=sd[:tl, :], in0=QPn[:tl, :], scalar=1.0,
                            in1=X[:tl, :], op0=OP.mult, op1=OP.mult,
                            accum_out=Smom[:tl, 0, d : d + 1],
                        )

                # ---- scale moments by polynomial coefficients (one tiny TT)
                A2 = mom.tile([128, 2, D + 1], F32, tag="A2")
                nc.vector.tensor_mul(A2[:tl, :, :], Smom[:tl, :, :], ctile[:tl, :, :])

                if CFG["phase_limit"] == 2:
                    O = io.tile([128, H], F32, tag="O")
                    nc.vector.tensor_copy(O[:tl, :], K)
                    nc.vector.tensor_scalar(
                        out=O[:tl, 0 : 2 * (D + 1)],
                        in0=A2[:tl, :, :].rearrange("p a b -> p (a b)"),
                        scalar1=1.0, scalar2=None, op0=OP.mult,
                    )
                    out_eng.dma_start(out=out[t0 : t0 + tl, :], in_=O[:tl, :])
                    continue

                # ---- K powers for Estrin: k^2, k^4, k^8
                if CFG["chain_mode"] == "estrin":
                    kp_engs = [nc.vector] * CFG["kpow_dve"] + [nc.gpsimd] * 3
                    K2 = pows.tile([128, H], F32, tag="K2")
                    kp_engs[0].tensor_mul(K2[:tl, :], K, K)
                    K4 = pows.tile([128, H], F32, tag="K4")
                    kp_engs[1].tensor_mul(K4[:tl, :], K2[:tl, :], K2[:tl, :])
                    K8 = pows.tile([128, H], F32, tag="K8")
                    kp_engs[2].tensor_mul(K8[:tl, :], K4[:tl, :], K4[:tl, :])

                # ---- Estrin evaluation of both polynomials over K
                # P(k) = (a0 + a1 k) + k^2 (a2 + a3 k)
                #      + k^4 [(a4 + a5 k) + k^2 (a6 + a7 k)] + a8 k^8
                cnt = {"pair": 0, "tt": 0}

                def estrin(which, tag):
                    a = lambda d: A2[:tl, which, d : d + 1]
                    ps = []
                    for i in range(4):
                        p = scrp.tile([128, H], F32, tag=f"p{tag}{i}")
                        if cnt["pair"] < CFG["pairs_act"]:
                            cnt["pair"] += 1
                            nc.scalar.activation(
                                p[:tl, :], K, AF.Identity,
                                scale=a(2 * i + 1), bias=a(2 * i),
                            )
                        else:
                            nc.vector.tensor_scalar(
                                out=p[:tl, :], in0=K, scalar1=a(2 * i + 1),
                                scalar2=a(2 * i), op0=OP.mult, op1=OP.add,
                            )
                        ps.append(p)
                    n_pool_tt = CFG["chain_tt_pool"]
                    engs = []
                    for _ in range(6):
                        engs.append(
                            nc.gpsimd if cnt["tt"] < n_pool_tt else nc.vector
                        )
                        cnt["tt"] += 1
                    t1 = scrp.tile([128, H], F32, tag=f"t1{tag}")
                    engs[0].tensor_mul(t1[:tl, :], ps[1][:tl, :], K2[:tl, :])
                    e01 = scrp.tile([128, H], F32, tag=f"e01{tag}")
                    engs[1].tensor_add(e01[:tl, :], t1[:tl, :], ps[0][:tl, :])
                    t2 = scrp.tile([128, H], F32, tag=f"t2{tag}")
                    engs[2].tensor_mul(t2[:tl, :], ps[3][:tl, :], K2[:tl, :])
                    e23 = scrp.tile([128, H], F32, tag=f"e23{tag}")
                    engs[3].tensor_add(e23[:tl, :], t2[:tl, :], ps[2][:tl, :])
                    t3 = scrp.tile([128, H], F32, tag=f"t3{tag}")
                    engs[4].tensor_mul(t3[:tl, :], e23[:tl, :], K4[:tl, :])
                    f = scrp.tile([128, H], F32, tag=f"f{tag}")
                    engs[5].tensor_add(f[:tl, :], t3[:tl, :], e01[:tl, :])
                    res = work.tile([128, H], F32, tag=f"res{tag}")
                    nc.vector.scalar_tensor_tensor(
                        out=res[:tl, :], in0=K8[:tl, :], scalar=a(8),
                        in1=f[:tl, :], op0=OP.mult, op1=OP.add,
                    )
                    return res

                def horner_chain(which, tag, add_eng, mul_eng, skip_final=False):
                    # u = a_D k; repeat: u = (u + a_d) * k; final +a_0
                    a = lambda d: A2[:tl, which, d : d + 1]
                    u = work.tile([128, H], F32, tag=f"res{tag}")
                    nc.vector.tensor_scalar(
                        out=u[:tl, :], in0=K, scalar1=a(D), scalar2=None,
                        op0=OP.mult,
                    )
                    for d in range(D - 1, 0, -1):
                        if add_eng is None:
                            nc.vector.scalar_tensor_tensor(
                                out=u[:tl, :], in0=u[:tl, :], scalar=a(d),
                                in1=K, op0=OP.add, op1=OP.mult,
                            )
                        else:
                            add_eng(u, a(d))
                            mul_eng.tensor_mul(u[:tl, :], u[:tl, :], K)
                    if not skip_final:
                        nc.vector.tensor_scalar(
                            out=u[:tl, :], in0=u[:tl, :], scalar1=a(0),
                            scalar2=None, op0=OP.add,
                        )
                    return u

                mode = CFG["chain_mode"]
                skip_a0 = {"skip": False}
                if mode == "estrin":
                    uN = estrin(0, "n")
                    uD = estrin(1, "d")
                elif mode == "horner_dve":
                    skip_a0["skip"] = True
                    uN = horner_chain(0, "n", None, None, skip_final=True)
                    uD = horner_chain(1, "d", None, None)
                else:  # horner_mix: numerator on DVE, denominator ACT/Pool
                    uN = horner_chain(0, "n", None, None)

                    def act_add(u, aap):
                        nc.scalar.activation(
                            out=u[:tl, :], in_=u[:tl, :], func=AF.Identity,
                            bias=aap,
                        )

                    uD = horner_chain(1, "d", act_add, nc.gpsimd)

                if CFG["phase_limit"] == 3:
                    O = io.tile([128, H], F32, tag="O")
                    nc.vector.tensor_add(O[:tl, :], uN[:tl, :], uD[:tl, :])
                    out_eng.dma_start(out=out[t0 : t0 + tl, :], in_=O[:tl, :])
                    continue

                # ---- out = num / den
                rD = work.tile([128, H], F32, tag="rD")
                if CFG["recip"] == "fast":
                    nc.vector.reciprocal_approx_fast(rD[:tl, :], uD[:tl, :])
                elif CFG["recip"] == "approx":
                    rs = scrp.tile([128, H], F32, tag="scr")
                    nc.vector.reciprocal_approx_accurate(
                        rD[:tl, :], uD[:tl, :], rs[:tl, :]
                    )
                else:
                    nc.vector.reciprocal(rD[:tl, :], uD[:tl, :])
                O = io.tile([128, H], F32, tag="O")
                if skip_a0["skip"]:
                    # fused: out = (uN + a0_num) * (1/den)
                    nc.vector.scalar_tensor_tensor(
                        out=O[:tl, :], in0=uN[:tl, :],
                        scalar=A2[:tl, 0, 0:1], in1=rD[:tl, :],
                        op0=OP.add, op1=OP.mult,
                    )
                else:
                    fm_eng = nc.vector if CFG.get("fmul_dve") else nc.gpsimd
                    fm_eng.tensor_mul(O[:tl, :], uN[:tl, :], rD[:tl, :])
                out_eng.dma_start(out=out[t0 : t0 + tl, :], in_=O[:tl, :])

        if reps == 1:
            body()
        else:
            with tc.For_i(0, reps, 1):
                body()

    nc.compile()
    return nc


_NCS = {}


def _get_nc(with_bias: bool = True):
    if with_bias not in _NCS:
        _NCS[with_bias] = build_kernel(with_bias=with_bias)
    return _NCS[with_bias]


def _make_in_maps(x, W0, b0, W1, b1):
    coef = COEFS[D]
    xf = np.ascontiguousarray(np.asarray(x, np.float32).reshape(T, H))
    W0 = np.asarray(W0, np.float32)
    W1 = np.asarray(W1, np.float32)
    biasQ = np.zeros((128, H), np.float32)
    biasQ[0, :] = np.asarray(b1, np.float32)
    biasK = np.zeros((128, H), np.float32)
    biasK[0, :] = np.asarray(b0, np.float32)
    c2 = np.tile(
        np.array(coef + coef, np.float32).reshape(1, 2 * (D + 1)), (128, 1)
    )
    wcat = np.ascontiguousarray(
        np.concatenate(
            [W1[:128, :], W1[128:, :], biasQ, c2,
             W0[:128, :], W0[128:, :], biasK],
            axis=1,
        )
    )  # [128, WQ+WK]
    return [
        {
            "xs": np.ascontiguousarray(xf[c * TC : (c + 1) * TC]),
            "wcat": wcat,
        }
        for c in range(NCORES)
    ]


def _run(x, W0, b0, W1, b1, trace=False, **kw):
    with_bias = bool(
        np.any(np.asarray(b0, np.float32)) or np.any(np.asarray(b1, np.float32))
    )
    res = run_bass_kernel_spmd(
        _get_nc(with_bias), _make_in_maps(x, W0, b0, W1, b1),
        list(range(NCORES)), trace=trace, **kw,
    )
    outs = [res.results[c]["out"] for c in range(NCORES)]
    full = np.concatenate(outs, axis=0).reshape(B, S, M, H).astype(np.float32)
    return full, res


def kernel(x, W0, b0, W1, b1):
    full, _ = _run(x, W0, b0, W1, b1, trace=False)
    return full


# revision 66
# speedup vs baseline: 1.1430x; 1.0462x over previous
"""Trainium2 Bass kernel for per-token outer-product softmax attention.

Reference computation (per token t of 1600, H=256):
    k = tanh(x W0 + b0);  q = tanh(x W1 + b1)
    scores[i,j] = k[i]*q[j];  attn = softmax_j(scores);  out = attn @ x

Key algebra: k,q are tanh outputs so k[i]*q[j] in (-1,1). On [-1,1],
exp(s) is approximated to fp32-noise level by a low-degree polynomial
P(s) = sum_d c_d s^d, and P(k_i q_j) = sum_d c_d k_i^d q_j^d is
SEPARABLE. Softmax numerator/denominator become per-token moments:
    num_i = sum_d (c_d sum_j q_j^d x_j) k_i^d
    den_i = sum_d (c_d sum_j q_j^d)     k_i^d
so the 256x256 scores tensor is never materialized. Per 128-token tile
this is ~2D fused multiply+reduce passes (moments, via
scalar_tensor_tensor accum_out) plus two fused Horner chains over k,
all [128,256] vector instructions spread across DVE / GpSimd(Pool) /
ACT engines. The queries matmul+tanh is scheduled before the keys one
so the moment pipeline starts ASAP; the final +a0 of the numerator
chain is fused with the divide.

Sharding: pure data parallel over tokens, 200 tokens/core x 8 cores;
weights replicated.
"""

import numpy as np
from contextlib import ExitStack

import concourse.bass as bass
import concourse.bacc as bacc
import concourse.tile as tile
from concourse import mybir
from concourse.bass_utils import run_bass_kernel_spmd
from concourse.masks import make_identity

F32 = mybir.dt.float32
AF = mybir.ActivationFunctionType
OP = mybir.AluOpType

B, S, M, H = 4, 10, 40, 256
T = B * S * M            # 1600 tokens
NCORES = 8
TC = T // NCORES         # 200 tokens per core
BLOCKS = [(0, 128), (128, TC - 128)]

# Chebyshev-interpolation coefficients (monomial basis) of exp on [-1,1].
# Max rel err: D=6 -> 7.7e-6, D=8 -> 2.7e-8.
COEFS = {
    6: [1.0, 1.000022235, 0.5000027659, 0.1664890938, 0.04164456983,
        0.008686644402, 0.001432899535],
    8: [1.0, 0.9999999011, 0.4999999901, 0.1666679842, 0.04166679799,
        0.008328598904, 0.001388416857, 0.0002046983349, 2.542872193e-05],
}

D = 6

# Engine assignment knobs (tuned against real-HW loop benchmarks):
CFG = {
    "n_den_act": 6,     # denominator accums d=2..D: first n on ACT, rest DVE TS+accum
    "n_num_pool": 0,    # numerator moments d=2..D: first n via Pool TT + ACT accum
    "chain_tt_pool": 3,  # estrin only: of the 12 combine-TTs, how many on Pool
    "pairs_act": 8,     # estrin only: of the 8 pairs per block, how many on ACT
    "j0_act": True,     # d=0 numerator moment on ACT instead of DVE
    "tree_dve": 0,      # of the QP-tree TTs, how many on DVE instead of Pool
    "kpow_dve": 0,      # estrin only: of the 3 K-power TTs, how many on DVE
    "x_dma": "sync",    # engine for X loads: sync | scalar | gpsimd
    "out_dma": "sync",  # engine for output stores
    "recip": "approx",  # approx (~2 ULP custom DVE) | exact
    "scrp_bufs": 8,
    "phase_limit": 4,   # 0=min body, 1=KQ only, 2=+moments, 3=+chains, 4=full
    "chain_mode": "horner_dve",  # estrin | horner_dve | horner_mix
}


def _pow_tree(dmax):
    """Return list of (d, a, b) meaning QP_d = QP_a * QP_b, log-depth order."""
    steps = []
    have = {1}
    for d in range(2, dmax + 1):
        a = d // 2
        b = d - a
        steps.append((d, a, b))
        have.add(d)
    return steps


def build_kernel(reps: int = 1, with_bias: bool = True) -> bass.Bass:
    coef = COEFS[D]
    # wcat columns: [W1lo|W1hi|biasQ|coef || W0lo|W0hi|biasK]
    WQ = 2 * H + H + 2 * (D + 1)   # 786
    WK = 2 * H + H                 # 768
    WEXT = WQ + WK
    nc = bacc.Bacc("TRN2", target_bir_lowering=False, debug=False)
    xs = nc.declare_dram_parameter("xs", [TC, H], F32, isOutput=False)
    wcat = nc.declare_dram_parameter("wcat", [128, WEXT], F32, isOutput=False)
    out = nc.declare_dram_parameter("out", [TC, H], F32, isOutput=True)

    with tile.TileContext(nc) as tc, ExitStack() as ctx:
        consts = ctx.enter_context(tc.tile_pool(name="consts", bufs=1))
        io = ctx.enter_context(tc.tile_pool(name="io", bufs=CFG.get("io_bufs", 2)))
        work = ctx.enter_context(tc.tile_pool(name="work", bufs=CFG.get("work_bufs", 2)))
        pows = ctx.enter_context(tc.tile_pool(name="pows", bufs=CFG.get("pows_bufs", 2)))
        scrp = ctx.enter_context(tc.tile_pool(name="scrp", bufs=CFG.get("scrp_bufs", 3)))
        mom = ctx.enter_context(tc.tile_pool(name="mom", bufs=2))
        psT = ctx.enter_context(tc.tile_pool(name="psT", bufs=2, space="PSUM"))
        psKQ = ctx.enter_context(tc.tile_pool(name="psKQ", bufs=2, space="PSUM"))

        x_eng = getattr(nc, CFG["x_dma"])
        out_eng = getattr(nc, CFG["out_dma"])
        # Small constants first on the Pool queue, then X (gates the whole
        # pipeline), then the Q-side weights (gate MM-Q), then K-side.
        ident = consts.tile([128, 128], F32)
        make_identity(nc, ident)
        ones1 = consts.tile([1, 128], F32)
        nc.gpsimd.memset(ones1, 1.0)
        Xs = []
        for t0, tl in BLOCKS:
            X = io.tile([128, H], F32, tag=f"X{t0}")
            x_eng.dma_start(out=X[:tl, :], in_=xs[t0 : t0 + tl, :])
            Xs.append(X)
        wallQ = consts.tile([128, WQ], F32)
        nc.gpsimd.dma_start(out=wallQ, in_=wcat[:, 0:WQ])
        wallK = consts.tile([128, WK], F32)
        nc.gpsimd.dma_start(out=wallK, in_=wcat[:, WQ:WEXT])
        bsbQ = wallQ[0:1, 2 * H : 3 * H]
        bsbK = wallK[0:1, 2 * H : 3 * H]
        ctile = wallQ[:, 3 * H : 3 * H + 2 * (D + 1)].rearrange(
            "p (two d) -> p two d", two=2
        )

        def body():
            if CFG["phase_limit"] == 0:
                for t0, tl in BLOCKS:
                    O = io.tile([128, H], F32, tag="O")
                    nc.vector.tensor_copy(O[:tl, :], Xs[0][:tl, :])
                    out_eng.dma_start(out=out[t0 : t0 + tl, :], in_=O[:tl, :])
                return
            for bi, (t0, tl) in enumerate(BLOCKS):
                X = Xs[bi]

                # ---- x^T via PE transpose (matmul contracts over partitions)
                pT = psT.tile([128, 2, 128], F32, tag="pT")
                nc.tensor.transpose(pT[:, 0, :tl], X[:tl, 0:128], ident[:tl, :tl])
                nc.tensor.transpose(pT[:, 1, :tl], X[:tl, 128:256], ident[:tl, :tl])
                xT = work.tile([128, 2, 128], F32, tag="xT")
                nc.vector.tensor_copy(xT[:, :, :tl], pT[:, :, :tl])

                # ---- queries first: moments only need Q and X.
                # Bias matmul leads: it only needs constants, so it runs
                # during the xT dependency chain.
                psQ = psKQ.tile([128, H], F32, tag="psQ")
                if with_bias:
                    nc.tensor.matmul(
                        psQ[:tl, :], ones1[:, :tl], bsbQ,
                        start=True, stop=False,
                    )
                nc.tensor.matmul(
                    psQ[:tl, :], xT[:, 0, :tl], wallQ[:, 0:256],
                    start=not with_bias, stop=False,
                )
                nc.tensor.matmul(
                    psQ[:tl, :], xT[:, 1, :tl], wallQ[:, 256:512],
                    start=False, stop=True,
                )
                # Smom[:, 0, :] = raw numerator moments, [:, 1, :] = denominator
                Smom = mom.tile([128, 2, D + 1], F32, tag="Smom")
                nc.gpsimd.memset(Smom[:tl, 1, 0:1], float(H))
                Qt = work.tile([128, H], F32, tag="Qt")
                nc.scalar.activation(
                    Qt[:tl, :], psQ[:tl, :], AF.Tanh,
                    accum_out=Smom[:tl, 1, 1:2],
                )
                Q = Qt[:tl, :]

                # ---- keys (overlaps with the moment pipeline below)
                psK = psKQ.tile([128, H], F32, tag="psK")
                if with_bias:
                    nc.tensor.matmul(
                        psK[:tl, :], ones1[:, :tl], bsbK,
                        start=True, stop=False,
                    )
                nc.tensor.matmul(
                    psK[:tl, :], xT[:, 0, :tl], wallK[:, 0:256],
                    start=not with_bias, stop=False,
                )
                nc.tensor.matmul(
                    psK[:tl, :], xT[:, 1, :tl], wallK[:, 256:512],
                    start=False, stop=True,
                )
                Kt = work.tile([128, H], F32, tag="Kt")
                nc.scalar.activation(Kt[:tl, :], psK[:tl, :], AF.Tanh)
                K = Kt[:tl, :]

                if CFG["phase_limit"] == 1:
                    O = io.tile([128, H], F32, tag="O")
                    nc.vector.tensor_add(O[:tl, :], Qt[:tl, :], Kt[:tl, :])
                    out_eng.dma_start(out=out[t0 : t0 + tl, :], in_=O[:tl, :])
                    continue

                # ---- raw moments (unscaled powers QP_d = q^d)
                j0 = scrp.tile([128, H], F32, tag="scr")
                if CFG["j0_act"]:
                    nc.scalar.activation(
                        j0[:tl, :], X[:tl, :], AF.Identity,
                        accum_out=Smom[:tl, 0, 0:1],
                    )
                else:
                    nc.vector.tensor_scalar(
                        out=j0[:tl, :], in0=X[:tl, :], scalar1=1.0, scalar2=0.0,
                        op0=OP.mult, op1=OP.add, accum_out=Smom[:tl, 0, 0:1],
                    )
                s1 = scrp.tile([128, H], F32, tag="scr")
                nc.vector.scalar_tensor_tensor(
                    out=s1[:tl, :], in0=Q, scalar=1.0, in1=X[:tl, :],
                    op0=OP.mult, op1=OP.mult, accum_out=Smom[:tl, 0, 1:2],
                )
                QP = {1: Q}
                n_act = 0
                n_pool = 0
                n_tree_dve = 0
                for d, a, b in _pow_tree(D):
                    QPn = pows.tile([128, H], F32, tag=f"qp{d}")
                    if n_tree_dve < CFG["tree_dve"]:
                        n_tree_dve += 1
                        nc.vector.tensor_mul(QPn[:tl, :], QP[a], QP[b])
                    else:
                        nc.gpsimd.tensor_mul(QPn[:tl, :], QP[a], QP[b])
                    QP[d] = QPn[:tl, :]
                    # denominator accum
                    if n_act < CFG["n_den_act"]:
                        n_act += 1
                        ja = scrp.tile([128, H], F32, tag="scr")
                        nc.scalar.activation(
                            ja[:tl, :], QPn[:tl, :], AF.Identity,
                            accum_out=Smom[:tl, 1, d : d + 1],
                        )
                    elif CFG.get("den_dve_op", "ts") == "ts":
                        jr = scrp.tile([128, H], F32, tag="scr")
                        nc.vector.tensor_scalar(
                            out=jr[:tl, :], in0=QPn[:tl, :], scalar1=1.0,
                            scalar2=0.0, op0=OP.mult, op1=OP.add,
                            accum_out=Smom[:tl, 1, d : d + 1],
                        )
                    else:
                        nc.vector.tensor_reduce(
                            out=Smom[:tl, 1, d : d + 1], in_=QPn[:tl, :],
                            axis=mybir.AxisListType.X, op=OP.add,
                        )
                    # numerator moment: sum (q^d * x)
                    if n_pool < CFG["n_num_pool"]:
                        n_pool += 1
                        sd = scrp.tile([128, H], F32, tag="scr")
                        nc.gpsimd.tensor_mul(sd[:tl, :], QPn[:tl, :], X[:tl, :])
                        jb = scrp.tile([128, H], F32, tag="scr")
                        nc.scalar.activation(
                            jb[:tl, :], sd[:tl, :], AF.Identity,
                            accum_out=Smom[:tl, 0, d : d + 1],
                        )
                    else:
                        sd = scrp.tile([128, H], F32, tag="scr")
                        nc.vector.scalar_tensor_tensor(
                            out=sd[:tl, :], in0=QPn[:tl, :], scalar=1.0,
                            in1=X[:tl, :], op0=OP.mult, op1=OP.mult,
                            accum_out=Smom[:tl, 0, d : d + 1],
                        )

                # ---- scale moments by polynomial coefficients (one tiny TT)
                A2 = mom.tile([128, 2, D + 1], F32, tag="A2")
                nc.vector.tensor_mul(A2[:tl, :, :], Smom[:tl, :, :], ctile[:tl, :, :])

                if CFG["phase_limit"] == 2:
                    O = io.tile([128, H], F32, tag="O")
                    nc.vector.tensor_copy(O[:tl, :], K)
                    nc.vector.tensor_scalar(
                        out=O[:tl, 0 : 2 * (D + 1)],
                        in0=A2[:tl, :, :].rearrange("p a b -> p (a b)"),
                        scalar1=1.0, scalar2=None, op0=OP.mult,
                    )
                    out_eng.dma_start(out=out[t0 : t0 + tl, :], in_=O[:tl, :])
                    continue

                # ---- K powers for Estrin: k^2, k^4, k^8
                if CFG["chain_mode"] == "estrin":
                    kp_engs = [nc.vector] * CFG["kpow_dve"] + [nc.gpsimd] * 3
                    K2 = pows.tile([128, H], F32, tag="K2")
                    kp_engs[0].tensor_mul(K2[:tl, :], K, K)
                    K4 = pows.tile([128, H], F32, tag="K4")
                    kp_engs[1].tensor_mul(K4[:tl, :], K2[:tl, :], K2[:tl, :])
                    K8 = pows.tile([128, H], F32, tag="K8")
                    kp_engs[2].tensor_mul(K8[:tl, :], K4[:tl, :], K4[:tl, :])

                # ---- Estrin evaluation of both polynomials over K
                # P(k) = (a0 + a1 k) + k^2 (a2 + a3 k)
                #      + k^4 [(a4 + a5 k) + k^2 (a6 + a7 k)] + a8 k^8
                cnt = {"pair": 0, "tt": 0}

                def estrin(which, tag):
                    a = lambda d: A2[:tl, which, d : d + 1]
                    ps = []
                    for i in range(4):
                        p = scrp.tile([128, H], F32, tag=f"p{tag}{i}")
                        if cnt["pair"] < CFG["pairs_act"]:
                            cnt["pair"] += 1
                            nc.scalar.activation(
                                p[:tl, :], K, AF.Identity,
                                scale=a(2 * i + 1), bias=a(2 * i),
                            )
                        else:
                            nc.vector.tensor_scalar(
                                out=p[:tl, :], in0=K, scalar1=a(2 * i + 1),
                                scalar2=a(2 * i), op0=OP.mult, op1=OP.add,
                            )
                        ps.append(p)
                    n_pool_tt = CFG["chain_tt_pool"]
                    engs = []
                    for _ in range(6):
                        engs.append(
                            nc.gpsimd if cnt["tt"] < n_pool_tt else nc.vector
                        )
                        cnt["tt"] += 1
                    t1 = scrp.tile([128, H], F32, tag=f"t1{tag}")
                    engs[0].tensor_mul(t1[:tl, :], ps[1][:tl, :], K2[:tl, :])
                    e01 = scrp.tile([128, H], F32, tag=f"e01{tag}")
                    engs[1].tensor_add(e01[:tl, :], t1[:tl, :], ps[0][:tl, :])
                    t2 = scrp.tile([128, H], F32, tag=f"t2{tag}")
                    engs[2].tensor_mul(t2[:tl, :], ps[3][:tl, :], K2[:tl, :])
                    e23 = scrp.tile([128, H], F32, tag=f"e23{tag}")
                    engs[3].tensor_add(e23[:tl, :], t2[:tl, :], ps[2][:tl, :])
                    t3 = scrp.tile([128, H], F32, tag=f"t3{tag}")
                    engs[4].tensor_mul(t3[:tl, :], e23[:tl, :], K4[:tl, :])
                    f = scrp.tile([128, H], F32, tag=f"f{tag}")
                    engs[5].tensor_add(f[:tl, :], t3[:tl, :], e01[:tl, :])
                    res = work.tile([128, H], F32, tag=f"res{tag}")
                    nc.vector.scalar_tensor_tensor(
                        out=res[:tl, :], in0=K8[:tl, :], scalar=a(8),
                        in1=f[:tl, :], op0=OP.mult, op1=OP.add,
                    )
                    return res

                def horner_chain(which, tag, add_eng, mul_eng, skip_final=False):
                    # u = a_D k; repeat: u = (u + a_d) * k; final +a_0
                    a = lambda d: A2[:tl, which, d : d + 1]
                    u = work.tile([128, H], F32, tag=f"res{tag}")
                    nc.vector.tensor_scalar(
                        out=u[:tl, :], in0=K, scalar1=a(D), scalar2=None,
                        op0=OP.mult,
                    )
                    for d in range(D - 1, 0, -1):
                        if add_eng is None:
                            nc.vector.scalar_tensor_tensor(
                                out=u[:tl, :], in0=u[:tl, :], scalar=a(d),
                                in1=K, op0=OP.add, op1=OP.mult,
                            )
                        else:
                            add_eng(u, a(d))
                            mul_eng.tensor_mul(u[:tl, :], u[:tl, :], K)
                    if not skip_final:
                        nc.vector.tensor_scalar(
                            out=u[:tl, :], in0=u[:tl, :], scalar1=a(0),
                            scalar2=None, op0=OP.add,
                        )
                    return u

                mode = CFG["chain_mode"]
                skip_a0 = {"skip": False}
                if mode == "estrin":
                    uN = estrin(0, "n")
                    uD = estrin(1, "d")
                elif mode == "horner_dve":
                    skip_a0["skip"] = True
                    uN = horner_chain(0, "n", None, None, skip_final=True)
                    uD = horner_chain(1, "d", None, None)
                else:  # horner_mix: numerator on DVE, denominator ACT/Pool
                    uN = horner_chain(0, "n", None, None)

                    def act_add(u, aap):
                        nc.scalar.activation(
                            out=u[:tl, :], in_=u[:tl, :], func=AF.Identity,
                            bias=aap,
                        )

                    uD = horner_chain(1, "d", act_add, nc.gpsimd)

                if CFG["phase_limit"] == 3:
                    O = io.tile([128, H], F32, tag="O")
                    nc.vector.tensor_add(O[:tl, :], uN[:tl, :], uD[:tl, :])
                    out_eng.dma_start(out=out[t0 : t0 + tl, :], in_=O[:tl, :])
                    continue

                # ---- out = num / den
                rD = work.tile([128, H], F32, tag="rD")
                if CFG["recip"] == "fast":
                    nc.vector.reciprocal_approx_fast(rD[:tl, :], uD[:tl, :])
                elif CFG["recip"] == "approx":
                    rs = scrp.tile([128, H], F32, tag="scr")
                    nc.vector.reciprocal_approx_accurate(
                        rD[:tl, :], uD[:tl, :], rs[:tl, :]
                    )
                else:
                    nc.vector.reciprocal(rD[:tl, :], uD[:tl, :])
                O = io.tile([128, H], F32, tag="O")
                if skip_a0["skip"]:
                    # fused: out = (uN + a0_num) * (1/den)
                    nc.vector.scalar_tensor_tensor(
                        out=O[:tl, :], in0=uN[:tl, :],
                        scalar=A2[:tl, 0, 0:1], in1=rD[:tl, :],
                        op0=OP.add, op1=OP.mult,
                    )
                else:
                    fm_eng = nc.vector if CFG.get("fmul_dve") else nc.gpsimd
                    fm_eng.tensor_mul(O[:tl, :], uN[:tl, :], rD[:tl, :])
                out_eng.dma_start(out=out[t0 : t0 + tl, :], in_=O[:tl, :])

        if reps == 1:
            body()
        else:
            with tc.For_i(0, reps, 1):
                body()

    nc.compile()
    return nc


_NCS = {}


def _get_nc(with_bias: bool = True):
    if with_bias not in _NCS:
        _NCS[with_bias] = build_kernel(with_bias=with_bias)
    return _NCS[with_bias]


def _make_in_maps(x, W0, b0, W1, b1):
    coef = COEFS[D]
    xf = np.ascontiguousarray(np.asarray(x, np.float32).reshape(T, H))
    W0 = np.asarray(W0, np.float32)
    W1 = np.asarray(W1, np.float32)
    biasQ = np.zeros((128, H), np.float32)
    biasQ[0, :] = np.asarray(b1, np.float32)
    biasK = np.zeros((128, H), np.float32)
    biasK[0, :] = np.asarray(b0, np.float32)
    c2 = np.tile(
        np.array(coef + coef, np.float32).reshape(1, 2 * (D + 1)), (128, 1)
    )
    wcat = np.ascontiguousarray(
        np.concatenate(
            [W1[:128, :], W1[128:, :], biasQ, c2,
             W0[:128, :], W0[128:, :], biasK],
            axis=1,
        )
    )  # [128, WQ+WK]
    return [
        {
            "xs": np.ascontiguousarray(xf[c * TC : (c + 1) * TC]),
            "wcat": wcat,
        }
        for c in range(NCORES)
    ]


def _ensure_axon():
    # The PJRT path needs the axon devices as jax's default platform; if a
    # caller pinned cpu before importing us, try to restore axon.
    try:
        import jax
        if not any(d.platform == "axon" for d in jax.devices()):
            jax.config.update("jax_platforms", "axon,cpu")
    except Exception:
        pass


def _run(x, W0, b0, W1, b1, trace=False, **kw):
    _ensure_axon()
    with_bias = bool(
        np.any(np.asarray(b0, np.float32)) or np.any(np.asarray(b1, np.float32))
    )
    res = run_bass_kernel_spmd(
        _get_nc(with_bias), _make_in_maps(x, W0, b0, W1, b1),
        list(range(NCORES)), trace=trace, **kw,
    )
    outs = [res.results[c]["out"] for c in range(NCORES)]
    full = np.concatenate(outs, axis=0).reshape(B, S, M, H).astype(np.float32)
    return full, res


def kernel(x, W0, b0, W1, b1):
    full, _ = _run(x, W0, b0, W1, b1, trace=False)
    return full
